# revision 1
# baseline (speedup 1.0000x reference)
"""Trainium2 Bass kernel for nn_CrossAttention (dense_transformer).

Reference computation (per batch b, per stream s in {1,2}):
    q_s   = heads(x_s)                      # [H, N, D] slices of x_s
    kv_s  = x_s @ Wkv_s -> k_s, v_s         # [N, C] each
    gate_s= sigmoid(relu(x_s @ w1 + b1) @ w2 + b2)
    ctx_s = softmax_d( scale * k_s^T @ (v_s * gate_s) )   # [H, D, D], softmax over d
    o_1   = q_1 @ ctx_2 ; o_2 = q_2 @ ctx_1  (cross)

Sharding: 8 cores = (stream s, batch b) pairs.  Core (s, b) projects
x_s[b] (kv + gate + ctx_s[b]) and then computes the OTHER stream's
output o_{1-s}[b] = q_{1-s}[b] @ softmax(ctx_s[b]).  No cross-core
communication; host concatenates outputs.
"""

import numpy as np
from contextlib import ExitStack

N = 4096
C = 1024
H = 16
D = 64
SCALE = D ** (-0.5)
NCH = N // 128       # 32 n-chunks of 128 rows
KCH = C // 128       # 8 contraction chunks
F32 = None           # set lazily (mybir import)

_CACHE = {}


def _build_program(with_bias):
    """Build the SPMD Bass program (same for all 8 cores)."""
    import concourse.bass as bass
    import concourse.bacc as bacc
    import concourse.tile as tile
    import concourse.mybir as mybir

    F32 = mybir.dt.float32
    F32R = mybir.dt.float32r
    BF16 = mybir.dt.bfloat16
    AF = mybir.ActivationFunctionType

    nc = bacc.Bacc("TRN2", target_bir_lowering=False, debug=False, num_devices=8)

    xp = nc.dram_tensor("xp", [N, C], F32R, kind="ExternalInput").ap()
    xq = nc.dram_tensor("xq", [N, C], F32R, kind="ExternalInput").ap()
    wkv = nc.dram_tensor("wkv", [C, 2 * C], F32R, kind="ExternalInput").ap()
    w1 = nc.dram_tensor("w1", [C, C], F32R, kind="ExternalInput").ap()
    b1 = nc.dram_tensor("b1", [C], F32, kind="ExternalInput").ap()
    w2 = nc.dram_tensor("w2", [C, C], F32R, kind="ExternalInput").ap()
    b2 = nc.dram_tensor("b2", [C], F32R, kind="ExternalInput").ap()
    ident = nc.dram_tensor("ident", [128, 128], F32R, kind="ExternalInput").ap()
    identb = nc.dram_tensor("identb", [128, 128], BF16, kind="ExternalInput").ap()
    o = nc.dram_tensor("o", [N, C], F32R, kind="ExternalOutput").ap()


    with tile.TileContext(nc) as tc, ExitStack() as ctx:
        # ---------- persistent pools ----------
        cpool = ctx.enter_context(tc.tile_pool(name="consts", bufs=1))
        ident_sb = cpool.tile([128, 128], F32R, name="ident_sb")
        nc.sync.dma_start(ident_sb, ident)
        identf = cpool.tile([128, 128], F32, name="identf")
        nc.vector.tensor_copy(identf, ident_sb)
        identb_sb = cpool.tile([128, 128], BF16, name="identb_sb")
        nc.sync.dma_start(identb_sb, identb)
        b1_sb = cpool.tile([128, 8], F32, name="b1_sb")  # b1_sb[p, m] = b1[m*128+p]
        nc.sync.dma_start(b1_sb, b1.rearrange("(m p) -> p m", p=128))
        if with_bias:
            ones_sb = cpool.tile([1, 128], F32, name="ones_sb")
            nc.vector.memset(ones_sb, 1.0)
            ones_r = cpool.tile([1, 128], F32R, name="ones_r")
            nc.vector.tensor_copy(ones_r, ones_sb)
            b2_r = cpool.tile([1, C], F32R, name="b2_r")
            nc.sync.dma_start(b2_r, b2.rearrange("(one f) -> one f", one=1))

        acc_pool = ctx.enter_context(tc.tile_pool(name="ctxacc", bufs=1))
        # ctxT accumulator on partitions 0-63: head h -> cols [h*64, h*64+64), layout [e, d]
        ctx_acc = acc_pool.tile([64, 1024], F32, name="ctx_acc")
        nc.vector.memset(ctx_acc, 0.0)

        spool = ctx.enter_context(tc.tile_pool(name="spairs", bufs=1))
        spairs = [spool.tile([128, 128], BF16, name=f"spair{j}") for j in range(8)]

        dpool = ctx.enter_context(tc.tile_pool(name="scratch", bufs=1, space="DRAM"))
        g_dram = dpool.tile([N, C], F32, name="g_dram")
        xpT_dram = dpool.tile([C, N], F32R, name="xpT_dram")

        # =========================================================
        # Phase A1: gate MLP for all n; also builds/spills xp^T.
        #   gate1 transposed-out: hT[m-tile, n] = (xp @ w1).T  (w1 stationary)
        #   gate2 normal-out:     g[n, :] = sigmoid(h @ w2 + b2)  (hT stationary)
        # =========================================================
        with ExitStack() as a1:
            wpool = a1.enter_context(tc.tile_pool(name="a1w", bufs=1))
            w1_sb = wpool.tile([128, 8, C], F32R, name="w1_sb")  # [p, k, col]
            nc.sync.dma_start(w1_sb, w1.rearrange("(k p) m -> p k m", p=128))
            w2_sb = wpool.tile([128, 8, C], F32R, name="w2_sb")
            nc.sync.dma_start(w2_sb, w2.rearrange("(k p) m -> p k m", p=128))

            ht_pool = a1.enter_context(tc.tile_pool(name="a1ht", bufs=1))
            gout_pool = a1.enter_context(tc.tile_pool(name="a1g", bufs=1))
            g1ps_pool = a1.enter_context(
                tc.tile_pool(name="a1g1ps", bufs=4, space="PSUM")
            )
            g2ps_pool = a1.enter_context(
                tc.tile_pool(name="a1g2ps", bufs=2, space="PSUM")
            )

            def emit_transposes_g1(sb, xpt_pool, xin_pool, trps_pool):
                xpt = [
                    xpt_pool.tile([128, 1024], F32R, name=f"xpt{j}", tag=f"xpt{j}", bufs=1)
                    for j in range(8)
                ]
                for grp in range(2):  # 512-row halves
                    xins = []
                    for c4 in range(4):
                        xin = xin_pool.tile([128, C], F32R, name="xin", tag="xin")
                        nch = sb * 8 + grp * 4 + c4
                        nc.sync.dma_start(xin, xp[nch * 128:(nch + 1) * 128, :])
                        xins.append(xin)
                    for j in range(8):
                        tps = trps_pool.tile([128, 512], F32R, name="tps", tag="tps")
                        for c4 in range(4):
                            nc.tensor.transpose(
                                tps[:, c4 * 128:(c4 + 1) * 128],
                                xins[c4][:, j * 128:(j + 1) * 128],
                                ident_sb,
                            )
                        if j % 2 == 0:
                            nc.vector.tensor_copy(
                                xpt[j][:, grp * 512:(grp + 1) * 512], tps
                            )
                        else:
                            nc.scalar.copy(
                                xpt[j][:, grp * 512:(grp + 1) * 512], tps
                            )
                # spill xp^T
                for j in range(8):
                    nc.sync.dma_start(
                        xpT_dram[j * 128:(j + 1) * 128, sb * 1024:(sb + 1) * 1024],
                        xpt[j],
                    )
                # gate1 transposed: hT[m] = sum_k w1[k,m].T @ xpT[k]
                hts = [
                    ht_pool.tile([128, 1024], F32R, name=f"ht{m}", tag=f"ht{m}", bufs=2)
                    for m in range(8)
                ]
                for m in range(8):
                    pss = [
                        g1ps_pool.tile([128, 512], F32, name="g1ps", tag="g1ps")
                        for _ in range(2)
                    ]
                    for k in range(8):
                        lhs = w1_sb[:, k, m * 128:(m + 1) * 128]
                        for half in range(2):
                            nc.tensor.matmul(
                                pss[half],
                                lhs,
                                xpt[k][:, half * 512:(half + 1) * 512],
                                start=(k == 0),
                                stop=(k == 7),
                            )
                    for half in range(2):
                        nc.scalar.activation(
                            hts[m][:, half * 512:(half + 1) * 512],
                            pss[half],
                            AF.Relu,
                            bias=b1_sb[:, m:m + 1],
                        )
                return hts

            def emit_g2(sb, hts):
                for c in range(8):
                    nch = sb * 8 + c
                    gt = gout_pool.tile([128, C], F32, name="gt", tag="gt")
                    for t in range(2):
                        ps2 = g2ps_pool.tile([128, 512], F32, name="g2ps", tag="g2ps")
                        for k in range(8):
                            nc.tensor.matmul(
                                ps2,
                                hts[k][:, c * 128:(c + 1) * 128],
                                w2_sb[:, k, t * 512:(t + 1) * 512],
                                start=(k == 0),
                                stop=(k == 7 and not with_bias),
                            )
                        if with_bias:
                            nc.tensor.matmul(
                                ps2,
                                ones_r,
                                b2_r[:, t * 512:(t + 1) * 512],
                                start=False,
                                stop=True,
                            )
                        nc.scalar.activation(
                            gt[:, t * 512:(t + 1) * 512], ps2, AF.Sigmoid
                        )
                    nc.sync.dma_start(g_dram[nch * 128:(nch + 1) * 128, :], gt)

            with ExitStack() as a1inner:
                xpt_pool_i = a1inner.enter_context(tc.tile_pool(name="a1xpt", bufs=1))
                xin_pool_i = a1inner.enter_context(tc.tile_pool(name="a1xin", bufs=6))
                trps_pool_i = a1inner.enter_context(
                    tc.tile_pool(name="a1trps", bufs=2, space="PSUM")
                )
                for sb in range(3):
                    hts = emit_transposes_g1(sb, xpt_pool_i, xin_pool_i, trps_pool_i)
                    emit_g2(sb, hts)
                hts3 = emit_transposes_g1(3, xpt_pool_i, xin_pool_i, trps_pool_i)
            # xpt/xin/trps pools are now closed: A2's wkv tile will alias their
            # space, so its DMA can start while gate2(sb3) still runs on PE.
            emit_g2(3, hts3)

        # early phase-B pools: transposing xq is independent of A2/SM, so give
        # it non-aliased space and let the scheduler overlap it with A2/SM.
        bxin_pool = ctx.enter_context(tc.tile_pool(name="bxin", bufs=5))
        bxqt_pool = ctx.enter_context(tc.tile_pool(name="bxqt", bufs=1))
        btrps_early_pool = ctx.enter_context(
            tc.tile_pool(name="btrpse", bufs=2, space="PSUM")
        )
        bxqt_tiles = {}

        def emit_xq_transposes(blk):
            xins = []
            for c4 in range(4):
                xin = bxin_pool.tile([128, C], F32R, name="bxin", tag="bxin")
                nch = blk * 4 + c4
                nc.sync.dma_start(xin, xq[nch * 128:(nch + 1) * 128, :])
                xinb = bxin_pool.tile([128, C], BF16, name="bxinb", tag="bxinb")
                if c4 % 2 == 0:
                    nc.vector.tensor_copy(xinb, xin)
                else:
                    nc.scalar.copy(xinb, xin)
                xins.append(xinb)
            xqts = [
                bxqt_pool.tile(
                    [128, 512], BF16, name=f"xqt{j}", tag=f"xqt{j}", bufs=3
                )
                for j in range(8)
            ]
            for j in range(8):
                tps = btrps_early_pool.tile(
                    [128, 512], BF16, name="btps", tag="btps"
                )
                for c4 in range(4):
                    nc.tensor.transpose(
                        tps[:, c4 * 128:(c4 + 1) * 128],
                        xins[c4][:, j * 128:(j + 1) * 128],
                        identb_sb,
                    )
                if j % 2 == 0:
                    nc.vector.tensor_copy(xqts[j], tps)
                else:
                    nc.scalar.copy(xqts[j], tps)
            bxqt_tiles[blk] = xqts

        emit_xq_transposes(0)
        emit_xq_transposes(1)
        emit_xq_transposes(2)

        # =========================================================
        # Phase A2: kv projection + ctx accumulation.
        #   kv normal-out (xpT stationary); ctxT_h += vg_h.T @ k_h
        # =========================================================
        with ExitStack() as a2:
            wkv_pool = a2.enter_context(tc.tile_pool(name="a2w", bufs=1))
            wkv_sb = wkv_pool.tile([128, 8, 2 * C], F32R, name="wkv_sb")
            nc.sync.dma_start(wkv_sb, wkv.rearrange("(k p) m -> p k m", p=128))

            xpt_in_pool = a2.enter_context(tc.tile_pool(name="a2xpt", bufs=3))
            gin_pool = a2.enter_context(tc.tile_pool(name="a2gin", bufs=3))
            k_pool = a2.enter_context(tc.tile_pool(name="a2k", bufs=2))
            v_pool = a2.enter_context(tc.tile_pool(name="a2v", bufs=2))
            vg_pool = a2.enter_context(tc.tile_pool(name="a2vg", bufs=2))
            kvps_pool = a2.enter_context(
                tc.tile_pool(name="a2kvps", bufs=4, space="PSUM")
            )
            ctps_pool = a2.enter_context(
                tc.tile_pool(name="a2ctps", bufs=1, space="PSUM")
            )

            for nch in range(NCH):
                xpt_in = xpt_in_pool.tile([128, C], F32R, name="xpt_in", tag="xpt_in")
                nc.sync.dma_start(
                    xpt_in,
                    xpT_dram.rearrange("(k p) n -> p k n", p=128)[
                        :, :, nch * 128:(nch + 1) * 128
                    ],
                )
                gin = gin_pool.tile([128, C], F32, name="gin", tag="gin")
                nc.sync.dma_start(gin, g_dram[nch * 128:(nch + 1) * 128, :])

                kvps = [
                    kvps_pool.tile([128, 512], F32, name="kvps", tag="kvps")
                    for _ in range(4)
                ]
                for k in range(8):
                    lhs = xpt_in[:, k * 128:(k + 1) * 128]
                    for t in range(4):
                        nc.tensor.matmul(
                            kvps[t],
                            lhs,
                            wkv_sb[:, k, t * 512:(t + 1) * 512],
                            start=(k == 0),
                            stop=(k == 7),
                        )
                k_sb = k_pool.tile([128, C], F32R, name="k_sb", tag="k_sb")
                v_sb = v_pool.tile([128, C], F32, name="v_sb", tag="v_sb")
                nc.scalar.copy(k_sb[:, 0:512], kvps[0])
                nc.scalar.copy(k_sb[:, 512:1024], kvps[1])
                nc.vector.tensor_copy(v_sb[:, 0:512], kvps[2])
                nc.vector.tensor_copy(v_sb[:, 512:1024], kvps[3])
                vg = vg_pool.tile([128, C], F32R, name="vg", tag="vg")
                nc.vector.tensor_mul(vg, v_sb, gin)

                ctp = ctps_pool.tile([64, 1024], F32, name="ctp", tag="ctp")
                for h in range(H):
                    nc.tensor.matmul(
                        ctp[:, h * D:(h + 1) * D],
                        vg[:, h * D:(h + 1) * D],
                        k_sb[:, h * D:(h + 1) * D],
                        start=True,
                        stop=True,
                        skip_group_check=True,
                    )
                nc.vector.tensor_add(ctx_acc, ctx_acc, ctp)

        # =========================================================
        # Softmax over d (free dim of ctxT) + build block-diag S pairs
        # =========================================================
        with ExitStack() as sm:
            smp = sm.enter_context(tc.tile_pool(name="smpool", bufs=1))
            smps = sm.enter_context(tc.tile_pool(name="smps", bufs=2, space="PSUM"))
            maxs = smp.tile([64, 16], F32, name="maxs")
            nc.vector.tensor_reduce(
                maxs,
                ctx_acc.rearrange("p (b d) -> p b d", b=16),
                axis=mybir.AxisListType.X,
                op=mybir.AluOpType.max,
            )
            cmx = smp.tile([64, 1024], F32, name="cmx")
            nc.vector.tensor_sub(
                cmx.rearrange("p (h d) -> p h d", h=16),
                ctx_acc.rearrange("p (h d) -> p h d", h=16),
                maxs.unsqueeze(-1).broadcast_to([64, 16, 64]),
            )
            et = smp.tile([64, 1024], F32, name="et")
            nc.scalar.activation(et, cmx, AF.Exp, scale=float(SCALE))
            sums = smp.tile([64, 16], F32, name="sums")
            nc.vector.tensor_reduce(
                sums,
                et.rearrange("p (b d) -> p b d", b=16),
                axis=mybir.AxisListType.X,
                op=mybir.AluOpType.add,
            )
            recs = smp.tile([64, 16], F32, name="recs")
            nc.vector.reciprocal(recs, sums)
            st = smp.tile([64, 1024], F32, name="st")
            nc.vector.tensor_mul(
                st.rearrange("p (h d) -> p h d", h=16),
                et.rearrange("p (h d) -> p h d", h=16),
                recs.unsqueeze(-1).broadcast_to([64, 16, 64]),
            )
            # st: softmaxed ctxT [e, d] per head at cols h*64.  Transposing the
            # side-by-side pair [ctxT_2j | ctxT_2j+1] ([64, 128]) gives
            # [S_2j stacked above S_2j+1] ([128, 64]); scatter to block-diag.
            zero_sb = smp.tile([128, 128], BF16, name="zero_sb")
            nc.vector.memset(zero_sb, 0.0)
            for j in range(8):
                tp = smps.tile([128, 64], F32, name="smtp", tag="smtp")
                nc.tensor.transpose(
                    tp, st[:, (2 * j) * 64:(2 * j + 2) * 64], identf[0:64, 0:64]
                )
                nc.vector.tensor_copy(spairs[j], zero_sb)
                nc.vector.tensor_copy(spairs[j][0:64, 0:64], tp[0:64, :])
                nc.vector.tensor_copy(spairs[j][64:128, 64:128], tp[64:128, :])

        # =========================================================
        # Phase B: o[nchunk, j*128:(j+1)*128] = (xqT_j_chunk).T @ spair_j
        # (normal orientation directly; no back-transposes)
        # =========================================================
        with ExitStack() as pb:
            oout_pool = pb.enter_context(tc.tile_pool(name="bo", bufs=6))
            bops_pool = pb.enter_context(tc.tile_pool(name="bops", bufs=4, space="PSUM"))

            for blk in range(8):
                if blk + 3 < 8:
                    emit_xq_transposes(blk + 3)
                xqts = bxqt_tiles.pop(blk)
                oouts = [
                    oout_pool.tile([128, C], F32R, name="oo", tag="oo")
                    for _ in range(4)
                ]
                for c4 in range(4):
                    for half in range(2):
                        ops = bops_pool.tile([128, 512], F32, name="ops", tag="ops")
                        for jj in range(4):
                            j = half * 4 + jj
                            nc.tensor.matmul(
                                ops[:, jj * 128:(jj + 1) * 128],
                                xqts[j][:, c4 * 128:(c4 + 1) * 128],
                                spairs[j],
                                start=True,
                                stop=True,
                                skip_group_check=True,
                            )
                        if half == 0:
                            nc.vector.tensor_copy(
                                oouts[c4][:, half * 512:(half + 1) * 512], ops
                            )
                        else:
                            nc.scalar.copy(
                                oouts[c4][:, half * 512:(half + 1) * 512], ops
                            )
                for c4 in range(4):
                    nch = blk * 4 + c4
                    nc.sync.dma_start(o[nch * 128:(nch + 1) * 128, :], oouts[c4])

    nc.compile()
    return nc


def _get_program(with_bias=False):
    key = ("nc", bool(with_bias))
    if key not in _CACHE:
        _CACHE[key] = _build_program(with_bias)
    return _CACHE[key]


def make_in_maps(x1, x2, Wkv1, Wkv2, g1_w1, g1_b1, g1_w2, g1_b2,
                 g2_w1, g2_b1, g2_w2, g2_b2):
    """Core (s, b): cores 0-3 = (s=0, b), cores 4-7 = (s=1, b)."""
    import ml_dtypes
    ident = np.eye(128, dtype=np.float32)
    identb = np.eye(128, dtype=ml_dtypes.bfloat16)
    asf = np.ascontiguousarray
    in_maps = []
    for core in range(8):
        s, b = core // 4, core % 4
        if s == 0:
            m = dict(xp=asf(x1[b]), xq=asf(x2[b]), wkv=asf(Wkv1),
                     w1=asf(g1_w1), b1=asf(g1_b1), w2=asf(g1_w2), b2=asf(g1_b2))
        else:
            m = dict(xp=asf(x2[b]), xq=asf(x1[b]), wkv=asf(Wkv2),
                     w1=asf(g2_w1), b1=asf(g2_b1), w2=asf(g2_w2), b2=asf(g2_b2))
        m["ident"] = ident
        m["identb"] = identb
        in_maps.append(m)
    return in_maps


def kernel(x1, x2, Wkv1, Wkv2, g1_w1, g1_b1, g1_w2, g1_b2,
           g2_w1, g2_b1, g2_w2, g2_b2, _runner=None):
    """Full-input entry point.  Returns (o1, o2), each [4, 4096, 1024] f32."""
    from concourse.bass_utils import run_bass_kernel_spmd

    args = [np.asarray(a, dtype=np.float32) for a in
            (x1, x2, Wkv1, Wkv2, g1_w1, g1_b1, g1_w2, g1_b2,
             g2_w1, g2_b1, g2_w2, g2_b2)]
    with_bias = bool(np.any(args[7]) or np.any(args[11]))  # g1_b2, g2_b2
    nc = _get_program(with_bias)
    in_maps = make_in_maps(*args)
    if _runner is None:
        res = run_bass_kernel_spmd(nc, in_maps, core_ids=list(range(8)))
        results = res.results
    else:
        results = _runner(nc, in_maps)

    B = x1.shape[0]
    o1 = np.empty((B, N, C), dtype=np.float32)
    o2 = np.empty((B, N, C), dtype=np.float32)
    for core in range(8):
        s, b = core // 4, core % 4
        out = results[core]["o"]
        if s == 0:
            o2[b] = out   # core projected x1 -> ctx1 -> o2 = q2 @ ctx1
        else:
            o1[b] = out
    return (o1, o2)



# revision 4
# speedup vs baseline: 1.3686x; 1.3686x over previous
"""Trainium2 Bass kernel for nn_CrossAttention (dense_transformer).

Reference computation (per batch b, per stream s in {1,2}):
    q_s   = heads(x_s)                      # [H, N, D] slices of x_s
    kv_s  = x_s @ Wkv_s -> k_s, v_s         # [N, C] each
    gate_s= sigmoid(relu(x_s @ w1 + b1) @ w2 + b2)
    ctx_s = softmax_d( scale * k_s^T @ (v_s * gate_s) )   # [H, D, D], softmax over d
    o_1   = q_1 @ ctx_2 ; o_2 = q_2 @ ctx_1  (cross)

Sharding: 8 cores = (stream s, batch b) pairs.  Core (s, b) projects
x_s[b] (kv + gate + ctx_s[b]) and then computes the OTHER stream's
output o_{1-s}[b] = q_{1-s}[b] @ softmax(ctx_s[b]).  No cross-core
communication; host concatenates outputs.

v2: host pre-transposes and fp16-casts x (so no on-chip transposes),
fp16 matmul operands everywhere (fp32 PSUM accumulate), fused
block-pipelined gate1 -> gate2/kv/vg -> ctx (PSUM-accumulated across
all 32 n-chunks), softmax, then phase B streaming xqT.
"""

import numpy as np
from contextlib import ExitStack

N = 4096
C = 1024
H = 16
D = 64
SCALE = D ** (-0.5)
NBLK = 4            # n-blocks of 1024 rows
BN = N // NBLK      # 1024 rows per block
BCH = BN // 128     # 8 chunks of 128 rows per block

_CACHE = {}


def _build_program(with_bias):
    """Build the SPMD Bass program (same for all 8 cores)."""
    import concourse.bass as bass
    import concourse.bacc as bacc
    import concourse.tile as tile
    import concourse.mybir as mybir

    F32 = mybir.dt.float32
    F16 = mybir.dt.float16
    AF = mybir.ActivationFunctionType

    nc = bacc.Bacc("TRN2", target_bir_lowering=False, debug=False, num_devices=8)

    xpT = nc.dram_tensor("xpT", [C, N], F16, kind="ExternalInput").ap()
    xqT = nc.dram_tensor("xqT", [C, N], F16, kind="ExternalInput").ap()
    wkv = nc.dram_tensor("wkv", [C, 2 * C], F16, kind="ExternalInput").ap()
    w1 = nc.dram_tensor("w1", [C, C], F16, kind="ExternalInput").ap()
    w2 = nc.dram_tensor("w2", [C, C], F16, kind="ExternalInput").ap()
    b1 = nc.dram_tensor("b1", [C], F32, kind="ExternalInput").ap()
    b2 = nc.dram_tensor("b2", [C], F16, kind="ExternalInput").ap()
    identh = nc.dram_tensor("identh", [128, 128], F16, kind="ExternalInput").ap()
    maskh = nc.dram_tensor("maskh", [128, 128], F16, kind="ExternalInput").ap()
    o = nc.dram_tensor("o", [N, C], F16, kind="ExternalOutput").ap()

    with tile.TileContext(nc) as tc, ExitStack() as ctx:
        # ---------- persistent pools ----------
        cpool = ctx.enter_context(tc.tile_pool(name="consts", bufs=1))
        b1_sb = cpool.tile([128, 8], F32, name="b1_sb")  # b1_sb[p, m] = b1[m*128+p]
        nc.sync.dma_start(b1_sb, b1.rearrange("(m p) -> p m", p=128))
        identh_sb = cpool.tile([128, 128], F16, name="identh_sb")
        nc.sync.dma_start(identh_sb, identh)
        maskh_sb = cpool.tile([128, 128], F16, name="maskh_sb")
        nc.sync.dma_start(maskh_sb, maskh)
        if with_bias:
            ones_r = cpool.tile([1, 128], F16, name="ones_r")
            nc.vector.memset(ones_r, 1.0)
            b2_r = cpool.tile([1, C], F16, name="b2_r")
            nc.sync.dma_start(b2_r, b2.rearrange("(one f) -> one f", one=1))

        spool = ctx.enter_context(tc.tile_pool(name="spairs", bufs=1))
        spairs = [spool.tile([128, 128], F16, name=f"spair{j}") for j in range(8)]

        # weights: w1 first (gate1 is the critical path at startup)
        wpool = ctx.enter_context(tc.tile_pool(name="weights", bufs=1))
        w1_sb = wpool.tile([128, 8, C], F16, name="w1_sb")  # [p, k, m]
        nc.sync.dma_start(w1_sb, w1.rearrange("(k p) m -> p k m", p=128))

        # ctx^T accumulator in PSUM, [e(2-head pair), d(2-head pair)] per pair j
        # at cols j*128; accumulated across all 32 n-chunks via start/stop.
        ctxps_pool = ctx.enter_context(
            tc.tile_pool(name="ctxps", bufs=1, space="PSUM")
        )
        ctx_ps = ctxps_pool.tile([128, 1024], F32, name="ctx_ps")

        # xqT prefetch pool (phase B input), lives for whole program
        xqt_pool = ctx.enter_context(tc.tile_pool(name="xqt", bufs=2))
        xqt_tiles = {}

        def load_xqt(blk):
            t = xqt_pool.tile([128, 8, BN], F16, name="xqt", tag="xqt")
            nc.sync.dma_start(
                t,
                xqT.rearrange("(k p) n -> p k n", p=128)[
                    :, :, blk * BN:(blk + 1) * BN
                ],
            )
            xqt_tiles[blk] = t

        # =========================================================
        # Phase A: gate MLP + kv projection + ctx accumulation,
        # fused per n-block of 1024 rows.
        # =========================================================
        with ExitStack() as pa:
            xpt_pool = pa.enter_context(tc.tile_pool(name="xpt", bufs=2))
            xpt_tiles = {}

            def load_xpt(blk):
                t = xpt_pool.tile([128, 8, BN], F16, name="xpt", tag="xpt")
                nc.sync.dma_start(
                    t,
                    xpT.rearrange("(k p) n -> p k n", p=128)[
                        :, :, blk * BN:(blk + 1) * BN
                    ],
                )
                xpt_tiles[blk] = t

            load_xpt(0)
            # remaining big DMAs, in priority order behind xpt block 0
            wkv_sb = wpool.tile([128, 8, 2 * C], F16, name="wkv_sb")
            nc.sync.dma_start(wkv_sb, wkv.rearrange("(k p) m -> p k m", p=128))
            w2_sb = wpool.tile([128, 8, C], F16, name="w2_sb")
            nc.sync.dma_start(w2_sb, w2.rearrange("(k p) m -> p k m", p=128))
            load_xqt(0)
            load_xqt(1)

            ht_pool = pa.enter_context(tc.tile_pool(name="ht", bufs=2))
            g_pool = pa.enter_context(tc.tile_pool(name="g", bufs=3))
            k_pool = pa.enter_context(tc.tile_pool(name="k", bufs=3))
            vg_pool = pa.enter_context(tc.tile_pool(name="vg", bufs=3))
            g1ps_pool = pa.enter_context(
                tc.tile_pool(name="g1ps", bufs=2, space="PSUM")
            )
            g2ps_pool = pa.enter_context(
                tc.tile_pool(name="g2ps", bufs=2, space="PSUM")
            )
            kvps_pool = pa.enter_context(
                tc.tile_pool(name="kvps", bufs=2, space="PSUM")
            )

            # ctx matmuls are emitted one chunk late so their vector-produced
            # inputs (k, vg) are ready by the time PE reaches them.
            pending = []

            def emit_ctx():
                if not pending:
                    return
                k_sb, vg, nch = pending.pop(0)
                for j in range(8):
                    nc.tensor.matmul(
                        ctx_ps[:, j * 128:(j + 1) * 128],
                        vg[:, j * 128:(j + 1) * 128],
                        k_sb[:, j * 128:(j + 1) * 128],
                        start=(nch == 0),
                        stop=(nch == N // 128 - 1),
                        skip_group_check=True,
                    )

            for blk in range(NBLK):
                if blk + 1 < NBLK:
                    load_xpt(blk + 1)
                xpt = xpt_tiles.pop(blk)
                # gate1: hT[m, n] = relu((xp @ w1 + b1).T), w1 stationary
                ht = ht_pool.tile([128, 8, BN], F16, name="ht", tag="ht")
                for m in range(8):
                    for half in range(2):
                        ps = g1ps_pool.tile([128, 512], F32, name="g1ps", tag="g1ps")
                        for kk in range(8):
                            nc.tensor.matmul(
                                ps,
                                w1_sb[:, kk, m * 128:(m + 1) * 128],
                                xpt[:, kk, half * 512:(half + 1) * 512],
                                start=(kk == 0),
                                stop=(kk == 7),
                            )
                        nc.scalar.activation(
                            ht[:, m, half * 512:(half + 1) * 512],
                            ps,
                            AF.Relu,
                            bias=b1_sb[:, m:m + 1],
                        )
                for ch in range(BCH):
                    nch = blk * BCH + ch
                    # gate2: g[n, q] = sigmoid(h @ w2 + b2), hT stationary
                    g = g_pool.tile([128, C], F16, name="g", tag="g")
                    for half in range(2):
                        ps = g2ps_pool.tile([128, 512], F32, name="g2ps", tag="g2ps")
                        for kk in range(8):
                            nc.tensor.matmul(
                                ps,
                                ht[:, kk, ch * 128:(ch + 1) * 128],
                                w2_sb[:, kk, half * 512:(half + 1) * 512],
                                start=(kk == 0),
                                stop=(kk == 7 and not with_bias),
                            )
                        if with_bias:
                            nc.tensor.matmul(
                                ps,
                                ones_r,
                                b2_r[:, half * 512:(half + 1) * 512],
                                start=False,
                                stop=True,
                            )
                        nc.scalar.activation(
                            g[:, half * 512:(half + 1) * 512], ps, AF.Sigmoid
                        )
                    # kv projection: kv[n, m], xpT stationary
                    k_sb = k_pool.tile([128, C], F16, name="k_sb", tag="k_sb")
                    vg = vg_pool.tile([128, C], F16, name="vg", tag="vg")
                    for q in range(4):
                        ps = kvps_pool.tile([128, 512], F32, name="kvps", tag="kvps")
                        for kk in range(8):
                            nc.tensor.matmul(
                                ps,
                                xpt[:, kk, ch * 128:(ch + 1) * 128],
                                wkv_sb[:, kk, q * 512:(q + 1) * 512],
                                start=(kk == 0),
                                stop=(kk == 7),
                            )
                        if q < 2:
                            nc.vector.tensor_copy(k_sb[:, q * 512:(q + 1) * 512], ps)
                        else:
                            qq = q - 2
                            nc.vector.tensor_mul(
                                vg[:, qq * 512:(qq + 1) * 512],
                                ps,
                                g[:, qq * 512:(qq + 1) * 512],
                            )
                    emit_ctx()
                    pending.append((k_sb, vg, nch))
            emit_ctx()

        # =========================================================
        # Softmax over d (free dim of ctx^T) + build block-diag S pairs
        # =========================================================
        with ExitStack() as sm:
            smp = sm.enter_context(tc.tile_pool(name="smpool", bufs=1))
            smps = sm.enter_context(tc.tile_pool(name="smps", bufs=2, space="PSUM"))
            maxs = smp.tile([128, 16], F32, name="maxs")
            nc.vector.tensor_reduce(
                maxs,
                ctx_ps.rearrange("p (g d) -> p g d", g=16),
                axis=mybir.AxisListType.X,
                op=mybir.AluOpType.max,
            )
            cmx = smp.tile([128, 1024], F32, name="cmx")
            nc.vector.tensor_sub(
                cmx.rearrange("p (g d) -> p g d", g=16),
                ctx_ps.rearrange("p (g d) -> p g d", g=16),
                maxs.unsqueeze(-1).broadcast_to([128, 16, 64]),
            )
            et = smp.tile([128, 1024], F32, name="et")
            nc.scalar.activation(et, cmx, AF.Exp, scale=float(SCALE))
            sums = smp.tile([128, 16], F32, name="sums")
            nc.vector.tensor_reduce(
                sums,
                et.rearrange("p (g d) -> p g d", g=16),
                axis=mybir.AxisListType.X,
                op=mybir.AluOpType.add,
            )
            recs = smp.tile([128, 16], F32, name="recs")
            nc.vector.reciprocal(recs, sums)
            stb = smp.tile([128, 1024], F16, name="stb")
            nc.vector.tensor_mul(
                stb.rearrange("p (g d) -> p g d", g=16),
                et.rearrange("p (g d) -> p g d", g=16),
                recs.unsqueeze(-1).broadcast_to([128, 16, 64]),
            )
            # stb[:, j*128:(j+1)*128] = softmaxed ctxT pair [e(2), d(2)];
            # transpose -> [d(2), e(2)], mask off the off-diagonal garbage.
            for j in range(8):
                tp = smps.tile([128, 128], F16, name="smtp", tag="smtp")
                nc.tensor.transpose(
                    tp, stb[:, j * 128:(j + 1) * 128], identh_sb
                )
                nc.vector.tensor_mul(spairs[j], tp, maskh_sb)

        # =========================================================
        # Phase B: o[nchunk, j*128:(j+1)*128] = q_pair_chunk @ spair_j
        # lhsT = xqT rows of head-pair j (stationary), rhs = spair_j.
        # =========================================================
        with ExitStack() as pb:
            oo_pool = pb.enter_context(tc.tile_pool(name="oo", bufs=4))
            bops_pool = pb.enter_context(
                tc.tile_pool(name="bops", bufs=2, space="PSUM")
            )
            for blk in range(NBLK):
                if blk + 2 < NBLK:
                    load_xqt(blk + 2)
                xqt = xqt_tiles.pop(blk)
                for ch in range(BCH):
                    nch = blk * BCH + ch
                    ops = bops_pool.tile([128, 1024], F32, name="ops", tag="ops")
                    for j in range(8):
                        nc.tensor.matmul(
                            ops[:, j * 128:(j + 1) * 128],
                            xqt[:, j, ch * 128:(ch + 1) * 128],
                            spairs[j],
                            start=True,
                            stop=True,
                            skip_group_check=True,
                        )
                    oo = oo_pool.tile([128, C], F16, name="oo", tag="oo")
                    if ch % 2 == 0:
                        nc.vector.tensor_copy(oo, ops)
                    else:
                        nc.scalar.copy(oo, ops)
                    nc.sync.dma_start(o[nch * 128:(nch + 1) * 128, :], oo)

    nc.compile()
    return nc


def _get_program(with_bias=False):
    key = ("nc", bool(with_bias))
    if key not in _CACHE:
        _CACHE[key] = _build_program(with_bias)
    return _CACHE[key]


def make_in_maps(x1, x2, Wkv1, Wkv2, g1_w1, g1_b1, g1_w2, g1_b2,
                 g2_w1, g2_b1, g2_w2, g2_b2):
    """Core (s, b): cores 0-3 = (s=0, b), cores 4-7 = (s=1, b)."""
    f16 = np.float16
    ident = np.eye(128, dtype=f16)
    mask = np.zeros((128, 128), dtype=f16)
    mask[:64, :64] = np.float16(1.0)
    mask[64:, 64:] = np.float16(1.0)
    # transposed fp16 copies of each batch of each stream (shared across cores)
    x1T = [np.asarray(x1[b], np.float32).T.astype(f16) for b in range(x1.shape[0])]
    x2T = [np.asarray(x2[b], np.float32).T.astype(f16) for b in range(x2.shape[0])]
    wkv1h = np.asarray(Wkv1, np.float32).astype(f16)
    wkv2h = np.asarray(Wkv2, np.float32).astype(f16)
    w11h = np.asarray(g1_w1, np.float32).astype(f16)
    w12h = np.asarray(g1_w2, np.float32).astype(f16)
    w21h = np.asarray(g2_w1, np.float32).astype(f16)
    w22h = np.asarray(g2_w2, np.float32).astype(f16)
    b11 = np.asarray(g1_b1, np.float32)
    b21 = np.asarray(g2_b1, np.float32)
    b12h = np.asarray(g1_b2, np.float32).astype(f16)
    b22h = np.asarray(g2_b2, np.float32).astype(f16)
    in_maps = []
    for core in range(8):
        s, b = core // 4, core % 4
        if s == 0:
            m = dict(xpT=x1T[b], xqT=x2T[b], wkv=wkv1h,
                     w1=w11h, b1=b11, w2=w12h, b2=b12h)
        else:
            m = dict(xpT=x2T[b], xqT=x1T[b], wkv=wkv2h,
                     w1=w21h, b1=b21, w2=w22h, b2=b22h)
        m["identh"] = ident
        m["maskh"] = mask
        in_maps.append(m)
    return in_maps


def kernel(x1, x2, Wkv1, Wkv2, g1_w1, g1_b1, g1_w2, g1_b2,
           g2_w1, g2_b1, g2_w2, g2_b2, _runner=None):
    """Full-input entry point.  Returns (o1, o2), each [4, 4096, 1024] f32."""
    from concourse.bass_utils import run_bass_kernel_spmd

    args = [np.asarray(a, dtype=np.float32) for a in
            (x1, x2, Wkv1, Wkv2, g1_w1, g1_b1, g1_w2, g1_b2,
             g2_w1, g2_b1, g2_w2, g2_b2)]
    with_bias = bool(np.any(args[7]) or np.any(args[11]))  # g1_b2, g2_b2
    nc = _get_program(with_bias)
    in_maps = make_in_maps(*args)
    if _runner is None:
        res = run_bass_kernel_spmd(nc, in_maps, core_ids=list(range(8)))
        results = res.results
    else:
        results = _runner(nc, in_maps)

    B = x1.shape[0]
    o1 = np.empty((B, N, C), dtype=np.float32)
    o2 = np.empty((B, N, C), dtype=np.float32)
    for core in range(8):
        s, b = core // 4, core % 4
        out = np.asarray(results[core]["o"], dtype=np.float32)
        if s == 0:
            o2[b] = out   # core projected x1 -> ctx1 -> o2 = q2 @ ctx1
        else:
            o1[b] = out
    return (o1, o2)
